# revision 30
# baseline (speedup 1.0000x reference)
"""Trainium2 Bass kernel for nn_MixedAttention.

Full inputs in, full output out. Sharding: 8 cores = 2 (batch) x 4 (head
pairs). Each core computes 2 global + 2 local heads for one batch element.

Key algebraic rewrite for the local branch:
    lscores = (lq@lk1^T)@(lk1@lk2^T) = lq @ (lk1^T@lk1) @ lk2^T
with M = lk1^T@lk1 a [64,64] matrix -- turns a 2048^3 matmul chain into
two small matmuls plus one S x S matmul (30x less PE work).

v3 redesign (trace-driven, vs the 399us baseline and the 358us v2):
  * The PE's HAM clock gate (cold 1.2GHz / warm 2.4GHz) re-throttles
    whenever the PE micro-stalls; phases where PE-work-per-jc-slot only
    marginally exceeds the ACT Exp rate (1.2us per [128,1024] tile) sat
    cold for 85+us. The schedule therefore keeps the PE oversubscribed
    everywhere:
  * only wq/wk/wv projections run up front (A1a); the other four
    projections become filler chunks inside the global-attention units,
    raising PE work per jc-slot to ~2x the Exp rate.
  * when the filler queue runs dry, pump() pads with standalone bf16
    LDWEIGHTS of the identity -- a ~107ns PE-array op with no PSUM, no
    consumers, and no deps, purely to hold the HAM gate open (each real
    matmul re-loads its own weights, so the clobber is harmless).
  * the warmup before the first DMA-gated matmul is ~120 such LDWs.
  * pass-1 (local row max): natural-orientation bf16 score tiles
    [128,512] + DVE free-dim reduce_max, 64 units/head riding the
    filler queue; -max relayouts to qaug row 64 via a DRAM bounce.
  * ACT runs ONLY the 128 Exp tiles; DVE gets reduces/copies/bias-adds;
    output tails have priority over fillers (their PSUM/SBUF rings gate
    upstream units).
  * single PSUM scope: st 2x[128,1024] -> ACT, scratch ring 2x[128,512]
    -> DVE, ctx [65,1024]: exactly 8 banks.
"""

import math
import os
import sys
from collections import deque

import numpy as np

sys.path.insert(0, "/opt/trn_rl_repo")

B, S, HID, HEAD = 2, 2048, 1024, 64
SC = S // 128  # 16 s-chunks of 128
HC = HID // 128  # 8 hidden chunks
N_CORES = 8
SCALE = 1.0 / math.sqrt(HEAD)

W_NAMES = ["wq", "wk", "wv", "wlq", "wlk1", "wlk2", "wlv"]

_CACHE = {}
LAST_RESULTS = None  # stash of BassKernelResults for test.py profiling


def _build():
    import concourse.mybir as mybir
    import concourse.tile as tile
    from concourse import bacc
    from concourse.masks import make_identity

    f32 = mybir.dt.float32
    f32r = mybir.dt.float32r
    bf16 = mybir.dt.bfloat16
    AF = mybir.ActivationFunctionType
    ALU = mybir.AluOpType
    AX = mybir.AxisListType

    N_WARM = int(os.environ.get("N_WARM", "120"))
    PAD_LDW = int(os.environ.get("PAD_LDW", "4"))

    nc = bacc.Bacc("TRN2", target_bir_lowering=False, debug=False,
                   enable_asserts=False)

    hid_d = nc.dram_tensor("hid", (HID, S), f32r, kind="ExternalInput").ap()
    mask_d = nc.dram_tensor("mask", (S,), f32, kind="ExternalInput").ap()
    w_d = {n: nc.dram_tensor(n, (128, HC, 128), f32r,
                             kind="ExternalInput").ap()
           for n in W_NAMES}
    b_d = {n: nc.dram_tensor("b" + n[1:], (128,), f32,
                             kind="ExternalInput").ap() for n in W_NAMES}
    out_d = nc.dram_tensor("out", (S, 256), f32, kind="ExternalOutput").ap()

    dma_engines = None

    def dma_rr(i):
        return dma_engines[i % len(dma_engines)]

    with tile.TileContext(nc) as tc:
        dma_engines = [nc.sync, nc.gpsimd, nc.scalar]
        with (
            tc.tile_pool(name="const", bufs=1) as constp,
            tc.tile_pool(name="persist", bufs=1) as pp,
            tc.tile_pool(name="dramp", bufs=2, space="DRAM") as dramp,
            tc.tile_pool(name="psg", bufs=2, space="PSUM") as ps_g,
            tc.tile_pool(name="psp1", bufs=2, space="PSUM") as ps_p1,
            tc.tile_pool(name="psctx", bufs=1, space="PSUM") as ps_ctx,
        ):
            vgp = vp = ep = None  # bound after the A1a pool closes
            ident = constp.tile([128, 128], f32, name="ident")
            make_identity(nc, ident)
            ident_bf = constp.tile([128, 128], bf16, name="ident_bf")
            nc.vector.tensor_copy(ident_bf, ident)
            ident_r = constp.tile([128, 128], f32r, name="ident_r")
            nc.vector.tensor_copy(ident_r, ident)
            ones_sb = constp.tile([128, SC], f32, name="ones_sb")
            nc.vector.memset(ones_sb, 1.0)
            mask_sb = constp.tile([128, SC], f32, name="mask_sb")
            nc.gpsimd.dma_start(mask_sb,
                                mask_d.rearrange("(c p) -> p c", p=128))
            bias_sb = {}
            for n in W_NAMES:
                t = constp.tile([128, 1], f32, name=f"b_{n}")
                nc.gpsimd.dma_start(t, b_d[n][:, None])
                bias_sb[n] = t

            # persistent projection outputs (consumer dtypes)
            projT = {}
            for n in ("wq", "wk", "wv", "wlv"):
                projT[n] = pp.tile([128, S], bf16, name=f"projT_{n}")
            for n in ("wlq", "wlk1", "wlk2"):
                projT[n] = pp.tile([128, S], f32r, name=f"projT_{n}")
            # per-local-head views at their native base partitions (the
            # h=3 M matrix is built at PSUM partitions 64:128 via
            # tile_position so no staging copy of lq is needed)
            lqh = {2: projT["wlq"][0:64], 3: projT["wlq"][64:128]}
            k2a = {h: pp.tile([65, S], f32r, name=f"k2a_{h}") for h in (2, 3)}
            qaug_bf = {h: pp.tile([64, S], bf16, name=f"qaug_bf_{h}")
                       for h in (2, 3)}
            k2a_bf = {h: pp.tile([64, S], bf16, name=f"k2a_bf_{h}")
                      for h in (2, 3)}
            qaug = {h: pp.tile([65, S], f32r, name=f"qaug_{h}")
                    for h in (2, 3)}
            pmax = {h: pp.tile([128, SC, 4], f32r, name=f"pmax_{h}")
                    for h in (2, 3)}
            ones_row = constp.tile([1, S], f32, name="ones_row")
            nc.vector.memset(ones_row, 1.0)
            for h in (2, 3):
                nc.vector.tensor_copy(k2a[h][64:65, :], ones_row)

            # ---------- filler machinery ----------
            # tq: latency-sensitive tails (their rings gate upstream
            # units); fq: bulk filler manifest. Pads hold HAM open.
            fq = deque()
            tq = deque()
            nfq = [0]

            def pad():
                for _ in range(PAD_LDW):
                    nc.tensor.ldweights(ident_bf)

            def pump(k=1):
                for _ in range(k):
                    if tq:
                        tq.popleft()()
                    elif fq:
                        fq.popleft()()
                        nfq[0] += 1
                    else:
                        pad()

            def drain_fq_until(n):
                while nfq[0] < n and fq:
                    fq.popleft()()
                    nfq[0] += 1

            def drain_all():
                while tq or fq:
                    pump()

            # ---------- emission helpers ----------

            def build_vaug(vT_bf):
                # v natural [s, d] + ones column -> [128, SC, 65] bf16.
                # 8 transposes batch into one PSUM tile + one DVE copy.
                base = vT_bf.base_partition()
                idsl = slice(base, base + 64)
                vaug = vgp.tile([128, SC, 65], bf16, tag="vaug",
                                name="vaug", bufs=3)
                nc.vector.tensor_copy(vaug[:, :, 64], ones_sb)

                def group(g):
                    def run():
                        pt = ps_p1.tile([128, 8, 64], bf16, tag="p1",
                                        name="ptv")
                        for t8 in range(8):
                            t = g * 8 + t8
                            nc.tensor.transpose(
                                pt[:, t8], vT_bf[:, t * 128:(t + 1) * 128],
                                ident_bf[idsl, idsl])
                        nc.vector.tensor_copy(vaug[:, g * 8:(g + 1) * 8, :64],
                                              pt)
                    return run
                return vaug, [group(0), group(1)]

            def attention_pair(head, kT, qT, vaug, is_local, jp,
                               per_jc_pump=1):
                # pair-unit: 2 i-column blocks of 512; st -> exp -> ctx
                # (+denominators via the ones column), then transpose back
                # and divide by the sums.
                csl = slice(head * 64, (head + 1) * 64)
                i0 = jp * 1024
                ctx = ps_ctx.tile([65, 1024], f32, tag="ctx", name="ctx")

                def ctx_mm(jc, e):
                    nc.tensor.matmul(ctx[:, 0:512], lhsT=vaug[:, jc],
                                     rhs=e[:, 0:512],
                                     start=(jc == 0), stop=(jc == SC - 1))
                    nc.tensor.matmul(ctx[:, 512:1024], lhsT=vaug[:, jc],
                                     rhs=e[:, 512:1024],
                                     start=(jc == 0), stop=(jc == SC - 1))

                # one-jc software pipeline lag: the ctx matmuls for jc are
                # emitted after st/exp of jc+1, so the in-order PE queue
                # never waits on the Exp of the tile it just produced
                prev = None
                for jc in range(SC):
                    jsl = slice(jc * 128, (jc + 1) * 128)
                    st = ps_g.tile([128, 1024], f32, tag="g", name="st")
                    nc.tensor.matmul(st[:, 0:512], lhsT=kT[:, jsl],
                                     rhs=qT[:, i0:i0 + 512],
                                     start=True, stop=True)
                    nc.tensor.matmul(st[:, 512:1024], lhsT=kT[:, jsl],
                                     rhs=qT[:, i0 + 512:i0 + 1024],
                                     start=True, stop=True)
                    e = ep.tile([128, 1024], bf16, tag="e", name="e")
                    bias = 0.0 if is_local else mask_sb[:, jc:jc + 1]
                    nc.scalar.activation(e, st, AF.Exp, bias=bias,
                                         scale=SCALE)
                    pump(per_jc_pump)
                    if prev is not None:
                        ctx_mm(*prev)
                    prev = (jc, e)
                ctx_mm(*prev)
                ctx_sbc = vp.tile([65, 1024], f32, tag="ctx_sbc",
                                  name="ctx_sbc", bufs=2)
                nc.vector.tensor_copy(ctx_sbc, ctx)

                def make_tg(tg):
                    def run():
                        pts = ps_p1.tile([128, 4, 128], f32, tag="p1",
                                         name="pts")
                        rec = vp.tile([128, 4], f32, tag="rec", name="rec",
                                      bufs=2)
                        ot = vp.tile([128, 4, 64], f32, tag="ot",
                                     name="ot", bufs=4)
                        for q in range(4):
                            tt = tg * 4 + q
                            nc.tensor.transpose(
                                pts[:, q, :65],
                                ctx_sbc[:, tt * 128:(tt + 1) * 128],
                                ident[:65, :65])
                            nc.vector.reciprocal(
                                rec[:, q:q + 1], pts[:, q, 64:65])
                        for q in range(4):
                            nc.vector.tensor_scalar_mul(
                                ot[:, q], pts[:, q, :64], rec[:, q:q + 1])
                        nc.sync.dma_start(
                            out_d.rearrange("(t p) c -> p t c", p=128)[
                                :, jp * 8 + tg * 4:jp * 8 + tg * 4 + 4, csl],
                            ot)
                    return run

                tq.extend([make_tg(0), make_tg(1)])

            def prep_chunks(h):
                # local-prep emitted as filler-sized closures
                rs = slice((h % 2) * 64, (h % 2) * 64 + 64)
                idsl = slice(rs.start, rs.start + 64)
                lk1T = projT["wlk1"][rs]
                lk1nat = vp.tile([128, SC, 64], f32r, tag="lk1nat",
                                 name="lk1nat", bufs=1)
                # h=3's M lives at partitions 64:128 so q_mm can consume
                # projT_wlq[64:128] without a base-partition staging copy
                m_full = vp.tile([128, 64], f32r, tag="m_sb", name="m_sb",
                                 bufs=2)
                m_sb = m_full[rs]

                def lk1_group(g):
                    def run():
                        pt = ps_p1.tile([128, 8, 64], f32r, tag="p1",
                                        name="ptk")
                        for t8 in range(8):
                            t = g * 8 + t8
                            nc.tensor.transpose(
                                pt[:, t8], lk1T[:, t * 128:(t + 1) * 128],
                                ident_r[idsl, idsl])
                        nc.vector.tensor_copy(
                            lk1nat[:, g * 8:(g + 1) * 8], pt)
                    return run

                def m_chunk():
                    # M = lk1^T @ lk1 [64, 64] accumulated over 16 t's at
                    # psum base 0; h=3's copy shifts to partitions 64:128
                    # via a tiny SBUF->SBUF DMA (engines are lane-locked)
                    st = ps_p1.tile([128, 512], f32, tag="p1", name="mps")
                    for t in range(SC):
                        nc.tensor.matmul(st[:64, :64], lhsT=lk1nat[:, t],
                                         rhs=lk1nat[:, t],
                                         start=(t == 0), stop=(t == SC - 1))
                    if rs.start == 0:
                        nc.vector.tensor_copy(m_sb, st[:64, :64])
                    else:
                        mtmp = vp.tile([64, 64], f32r, tag="mtmp",
                                       name="mtmp", bufs=1)
                        nc.vector.tensor_copy(mtmp, st[:64, :64])
                        nc.sync.dma_start(m_sb, mtmp)

                def q_mm(q):
                    def run():
                        mm = ps_p1.tile([128, 512], f32, tag="p1",
                                        name="mmq")
                        nc.tensor.matmul(
                            mm[:64, :], lhsT=m_sb,
                            rhs=lqh[h][:, q * 512:(q + 1) * 512],
                            start=True, stop=True)
                        nc.vector.tensor_copy(
                            qaug[h][:64, q * 512:(q + 1) * 512], mm[:64])
                        nc.vector.tensor_copy(
                            qaug_bf[h][:, q * 512:(q + 1) * 512], mm[:64])
                    return run

                def k2_shadow(half):
                    def run():
                        ssl = slice(half * 1024, (half + 1) * 1024)
                        nc.vector.tensor_copy(k2a_bf[h][:, ssl],
                                              k2a[h][0:64, ssl])
                    return run

                vaug, vgroups = build_vaug(projT["wlv"][rs])
                chunks = ([lk1_group(0), lk1_group(1), m_chunk,
                           k2_shadow(0), k2_shadow(1)]
                          + [q_mm(q) for q in range(4)] + vgroups)
                return vaug, chunks

            def pass1_unit(h, t, jq):
                # one [128,512] tile of raw local scores in the natural
                # orientation (i on partitions); DVE free-dim reduce_max
                def run():
                    tsl = slice(t * 128, (t + 1) * 128)
                    j0 = jq * 512
                    st = ps_p1.tile([128, 512], f32, tag="p1", name="st1")
                    nc.tensor.matmul(st, lhsT=qaug_bf[h][:, tsl],
                                     rhs=k2a_bf[h][:, j0:j0 + 512],
                                     start=True, stop=True)
                    nc.vector.tensor_reduce(pmax[h][:, t, jq:jq + 1], st,
                                            axis=AX.X, op=ALU.max)
                return run

            def pass1_combine(h):
                # fold the 4 j-quarter maxes, negate, and route [128, SC]
                # -> [1, S] via a DRAM roundtrip into qaug row 64
                def run():
                    maxneg = vp.tile([128, SC], f32r, tag="maxneg",
                                     name="maxneg", bufs=2)
                    nc.vector.tensor_reduce(maxneg, pmax[h], axis=AX.X,
                                            op=ALU.max, negate=True)
                    mscr = dramp.tile([S], f32r, tag="mscr", name="mscr")
                    nc.sync.dma_start(
                        mscr.rearrange("(t p) -> p t", p=128), maxneg)
                    nc.sync.dma_start(qaug[h][64:65, :], mscr[None, :])
                return run

            # ---------- A1a: wq/wk/wv projections, then attention with
            # the remaining projections riding the filler queue ----------
            with (
                tc.tile_pool(name="hidT", bufs=1) as hp,
                tc.tile_pool(name="io", bufs=1) as iop,
            ):
                iop_a_ctx = tc.tile_pool(name="ioa", bufs=1)
                iop_a = iop_a_ctx.__enter__()
                # HAM warmup across the input-DMA prologue
                for _ in range(N_WARM):
                    nc.tensor.ldweights(ident_bf)

                hidT = hp.tile([128, HC, S], f32r, name="hidT")
                hid_r = hid_d.rearrange("(c p) s -> p c s", p=128)
                dmai = 0
                wsbs = {}

                def emit_wdma(n):
                    nonlocal dmai
                    pool = iop_a if n in ("wq", "wk", "wv") else iop
                    wsb = pool.tile([128, HC, 128], f32r, tag=f"w_{n}",
                                    bufs=1, name=f"w_{n}")
                    dma_rr(dmai).dma_start(wsb, w_d[n])
                    dmai += 1
                    wsbs[n] = wsb

                def emit_hid_quarter(s4):
                    nonlocal dmai
                    ssl = slice(s4 * 512, (s4 + 1) * 512)
                    for hc in range(HC):
                        dma_rr(dmai).dma_start(hidT[:, hc, ssl],
                                               hid_r[:, hc, ssl])
                        dmai += 1

                emit_wdma("wq")
                emit_wdma("wk")
                emit_hid_quarter(0)
                emit_wdma("wv")
                emit_wdma("wlq")
                emit_hid_quarter(1)
                emit_wdma("wlk1")
                emit_wdma("wlk2")
                emit_wdma("wlv")
                emit_hid_quarter(2)
                emit_hid_quarter(3)

                def proj_quarter(n, s4):
                    ssl = slice(s4 * 512, (s4 + 1) * 512)
                    acc = ps_p1.tile([128, 512], f32, tag="p1", name="acc")
                    for hc in range(HC):
                        nc.tensor.matmul(
                            acc, lhsT=wsbs[n][:, hc],
                            rhs=hidT[:, hc, ssl],
                            start=(hc == 0), stop=(hc == HC - 1))
                    nc.vector.tensor_scalar_add(
                        projT[n][:, ssl], acc, bias_sb[n])
                    # per-head staging once a tile completes (partition
                    # shifts ride the DMA path; engines are lane-locked)
                    if n == "wlk2" and s4 == 3:
                        nc.sync.dma_start(k2a[2][0:64, :],
                                          projT["wlk2"][0:64, :])
                        nc.sync.dma_start(k2a[3][0:64, :],
                                          projT["wlk2"][64:128, :])

                for s4 in range(4):
                    for n in ("wq", "wk", "wv"):
                        proj_quarter(n, s4)
                    pump(1)
                iop_a_ctx.__exit__(None, None, None)

                with (
                    tc.tile_pool(name="vgpool", bufs=1) as vgp,
                    tc.tile_pool(name="vpool", bufs=1) as vp,
                    tc.tile_pool(name="epool", bufs=3) as ep,
                ):
                    # manifest: A1b projection quarters, then per-head
                    # prep / pass-1 so head 2's -max lands well before
                    # pass-2 h2
                    for n in ("wlq", "wlk1", "wlk2", "wlv"):
                        for s4 in range(4):
                            fq.append(lambda n=n, s4=s4: proj_quarter(n, s4))

                    gvaug0, gv0_chunks = build_vaug(projT["wv"][0:64])
                    gvaug1, gv1_chunks = build_vaug(projT["wv"][64:128])
                    for c in gv0_chunks:
                        c()
                    fq.extend(gv1_chunks)

                    lvaug2, prep2 = prep_chunks(2)
                    fq.extend(prep2)
                    for t in range(SC):
                        for jq in range(4):
                            fq.append(pass1_unit(2, t, jq))
                    fq.append(pass1_combine(2))
                    n_manifest_h2 = nfq[0] + len(fq)
                    lvaug3, prep3 = prep_chunks(3)
                    fq.extend(prep3)
                    for t in range(SC):
                        for jq in range(4):
                            fq.append(pass1_unit(3, t, jq))
                    fq.append(pass1_combine(3))
                    n_manifest_h3 = nfq[0] + len(fq)

                    # global attention: 4 units x 16 jc slots
                    for hh, jp in [(h, p)
                                   for h in range(2) for p in range(2)]:
                        rs = slice(hh * 64, (hh + 1) * 64)
                        attention_pair(hh, projT["wk"][rs], projT["wq"][rs],
                                       gvaug0 if hh == 0 else gvaug1,
                                       False, jp, per_jc_pump=2)
                    drain_fq_until(n_manifest_h2)

                    # ---------- pass-2 local attention ----------
                    attention_pair(2, k2a[2], qaug[2], lvaug2, True, 0,
                                   per_jc_pump=2)
                    attention_pair(2, k2a[2], qaug[2], lvaug2, True, 1,
                                   per_jc_pump=2)
                    drain_fq_until(n_manifest_h3)
                    attention_pair(3, k2a[3], qaug[3], lvaug3, True, 0,
                                   per_jc_pump=2)
                    attention_pair(3, k2a[3], qaug[3], lvaug3, True, 1,
                                   per_jc_pump=2)
                    drain_all()

    nc.compile()
    return nc


def kernel(**inputs):
    from concourse import bass_utils

    global LAST_RESULTS
    if "nc" not in _CACHE:
        _CACHE["nc"] = _build()
    nc = _CACHE["nc"]

    inputs = dict(inputs)
    inputs["wlv"] = np.asarray(inputs["wlv1"]) + np.asarray(inputs["wlv2"])
    inputs["blv"] = np.asarray(inputs["blv1"]) + np.asarray(inputs["blv2"])
    hs = np.ascontiguousarray(np.asarray(inputs["hidden_states"], np.float32))
    am = np.ascontiguousarray(np.asarray(inputs["attention_mask"], np.float32))
    in_maps = []
    for c in range(N_CORES):
        b, g = c // 4, c % 4
        csl = slice(128 * g, 128 * (g + 1))
        m = {"hid": np.ascontiguousarray(hs[b].T), "mask": am[b, 0, 0]}
        for n in W_NAMES:
            w = np.asarray(inputs[n], np.float32)[:, csl]
            m[n] = np.ascontiguousarray(
                w.reshape(HC, 128, 128).transpose(1, 0, 2))
            m["b" + n[1:]] = np.ascontiguousarray(
                np.asarray(inputs["b" + n[1:]], np.float32)[csl])
        in_maps.append(m)

    res = bass_utils.run_bass_kernel_spmd(
        nc, in_maps, list(range(N_CORES)),
        tmpdir=os.environ.get("BASS_TMPDIR"))
    LAST_RESULTS = res

    out = np.zeros((B, S, HID), np.float32)
    for c in range(N_CORES):
        b, g = c // 4, c % 4
        o = res.results[c]["out"]
        out[b, :, 128 * g:128 * (g + 1)] = o[:, :128]
        out[b, :, 512 + 128 * g:512 + 128 * (g + 1)] = o[:, 128:]
    return out
